# revision 1
# baseline (speedup 1.0000x reference)
"""Trainium2 Bass kernel for nn_DepthCue (dynamic-filter / CARAFE-style module).

Sharding: data-parallel over batch B=8 across the 8 NeuronCores (one sample
per core). Per core:
  - guide network (3x3 convs C->64->64->C) + DCK (1x1 convs + BN/ReLU) run on
    TensorE as shifted matmuls in float32r (1 cyc/row at N=512).
  - dynamic-filter apply: partitions = (h-band, group); per-tap elementwise
    multiply on VectorE in bf16 (filters broadcast across the 16 group
    channels via a step-0 AP dim); tap accumulation via identity-matmul into
    PSUM (fp32); residual added via an f32r identity-matmul PSUM init.
"""

import numpy as np
import ml_dtypes

import concourse.bass as bass
import concourse.bacc as bacc
import concourse.mybir as mybir
from concourse import bass_utils
from concourse.tile import TileContext

F32 = mybir.dt.float32
F32R = mybir.dt.float32r
BF16 = mybir.dt.bfloat16
MULT = mybir.AluOpType.mult
RELU = mybir.ActivationFunctionType.Relu
COPY = mybir.ActivationFunctionType.Copy

N_CORES = 8
C, H, W = 512, 64, 64
HID = 64          # guide-net hidden channels
RED = 128         # DCK reduction channels
G = 32            # groups
GC = 16           # channels per group
K = 7             # dynamic kernel size
NTAP = K * K      # 49
TPAD = 64         # taps padded to 64 in dck2 weights / fbuf
NB = 4            # h-bands (partition index = band*32 + g)
BH = 16           # rows per band
XH = BH + 6       # xb stored rows per band (halo 3 top + 3 bottom)
XW = W + 6        # xb stored cols (halo 3 + 3)
CST = XH * XW     # per-channel stride in xb free dim (1540)
PIX = H * W       # 4096
PW = W + 2        # padded width for conv intermediates (66)
PHW = (H + 2) * PW


def ap_of(t, offset, dims):
    """Raw AP over tile/dram tensor t: dims = [[step, count], ...] (dim0 = partition for sbuf)."""
    base = t if isinstance(t, bass.AP) else t[:]
    return bass.AP(tensor=base.tensor, offset=offset, ap=[list(d) for d in dims])


def build_nc():
    nc = bacc.Bacc(trn_type="TRN2", target_bir_lowering=False, debug=False)

    T = {}
    T["x_in"] = nc.dram_tensor("x", [C, H, W], F32, kind="ExternalInput").ap()
    T["xr"] = nc.dram_tensor("xr", [C, H, W], F32R, kind="ExternalInput").ap()
    T["w1t"] = nc.dram_tensor("w1t", [128, 9 * 4 * HID], F32R, kind="ExternalInput").ap()
    T["b1"] = nc.dram_tensor("b1", [HID, 1], F32, kind="ExternalInput").ap()
    T["w2t"] = nc.dram_tensor("w2t", [HID, 9 * HID], F32R, kind="ExternalInput").ap()
    T["b2"] = nc.dram_tensor("b2", [HID, 1], F32, kind="ExternalInput").ap()
    T["w3t"] = nc.dram_tensor("w3t", [HID, 9 * C], F32R, kind="ExternalInput").ap()
    T["b3"] = nc.dram_tensor("b3", [128, 4], F32, kind="ExternalInput").ap()
    T["dw1t"] = nc.dram_tensor("dw1t", [128, 4 * RED], F32R, kind="ExternalInput").ap()
    T["bnsc"] = nc.dram_tensor("bnsc", [RED, 1], F32, kind="ExternalInput").ap()
    T["bnsh"] = nc.dram_tensor("bnsh", [RED, 1], F32, kind="ExternalInput").ap()
    T["dw2t"] = nc.dram_tensor("dw2t", [RED, G * TPAD], F32R, kind="ExternalInput").ap()
    T["idb"] = nc.dram_tensor("idb", [128, 128], BF16, kind="ExternalInput").ap()
    T["out"] = nc.dram_tensor("out", [C, H, W], F32, kind="ExternalOutput").ap()
    # filters scratch: [band, g, tap(64), h(16), w] bf16
    T["fbuf"] = nc.dram_tensor("fbuf", [NB, G, TPAD, BH, W], BF16, kind="Internal").ap()

    with TileContext(nc) as tc:
        build_body(nc, tc, T)
    nc.compile()
    return nc


def conv_rhs(src, r0, tap, nh):
    dy, dx = tap // 3, tap % 3
    return ap_of(
        src, (r0 + dy) * PW + dx, [[PHW, src.shape[0]], [PW, nh], [1, W]]
    )


def build_body(nc, tc, T):
    x_in, out, fbuf = T["x_in"], T["out"], T["fbuf"]

    with tc.tile_pool(name="wpool", bufs=1) as wp:
        # ---- persistent weights ----
        w1s = wp.tile([128, 9 * 4 * HID], F32R)      # [ci%128, (tap, cc, co)]
        nc.sync.dma_start(w1s[:], T["w1t"][:])
        w2s = wp.tile([HID, 9 * HID], F32R)          # [ci, (tap, co)]
        nc.sync.dma_start(w2s[:], T["w2t"][:])
        w3s = wp.tile([HID, 9 * C], F32R)            # [ci, (tap, co)]
        nc.sync.dma_start(w3s[:], T["w3t"][:])
        dw1s = wp.tile([128, 4 * RED], F32R)         # [ci%128, (cc, co)]
        nc.sync.dma_start(dw1s[:], T["dw1t"][:])
        dw2s = wp.tile([RED, G * TPAD], F32R)
        nc.sync.dma_start(dw2s[:], T["dw2t"][:])
        b1s = wp.tile([HID, 1], F32)
        nc.sync.dma_start(b1s[:], T["b1"][:])
        b2s = wp.tile([HID, 1], F32)
        nc.sync.dma_start(b2s[:], T["b2"][:])
        b3s = wp.tile([128, 4], F32)
        nc.sync.dma_start(b3s[:], T["b3"][:])
        bnscs = wp.tile([RED, 1], F32)
        nc.sync.dma_start(bnscs[:], T["bnsc"][:])
        bnshs = wp.tile([RED, 1], F32)
        nc.sync.dma_start(bnshs[:], T["bnsh"][:])
        idbs = wp.tile([128, 128], BF16)
        nc.sync.dma_start(idbs[:], T["idb"][:])

        # ================= guide network + DCK =================
        with (
            tc.tile_pool(name="h12", bufs=1) as hp_,
            tc.tile_pool(name="cps", bufs=4, space="PSUM") as cps,
        ):
            h1 = hp_.tile([HID, PHW], F32R)
            nc.gpsimd.memset(h1[:].bitcast(F32), 0.0)
            h2 = hp_.tile([HID, PHW], F32R)
            nc.gpsimd.memset(h2[:].bitcast(F32), 0.0)

            with tc.tile_pool(name="xcp", bufs=1) as xcp:
                xc = []
                for cc in range(4):
                    t = xcp.tile([128, PHW], F32R, name=f"xc{cc}")
                    nc.gpsimd.memset(t[:].bitcast(F32), 0.0)
                    nc.sync.dma_start(
                        ap_of(t, PW + 1, [[PHW, 128], [PW, H], [1, W]]),
                        ap_of(T["xr"], cc * 128 * PIX, [[PIX, 128], [W, H], [1, W]]),
                    )
                    xc.append(t)

                # conv1: C->HID, 9 taps, 4 ci-chunks
                for oc in range(8):
                    ps = cps.tile([HID, 512], F32, tag="cv")
                    nmm = 0
                    for cc in range(4):
                        for tap in range(9):
                            nc.tensor.matmul(
                                ps[:],
                                w1s[:, (tap * 4 + cc) * HID:(tap * 4 + cc + 1) * HID],
                                conv_rhs(xc[cc], oc * 8, tap, 8),
                                start=(nmm == 0),
                                stop=(nmm == 35),
                            )
                            nmm += 1
                    nc.scalar.activation(
                        ap_of(h1, (oc * 8 + 1) * PW + 1, [[PHW, HID], [PW, 8], [1, W]]),
                        ps[:],
                        RELU,
                        bias=b1s[:],
                    )

            # conv2: HID->HID
            for oc in range(8):
                ps = cps.tile([HID, 512], F32, tag="cv")
                for tap in range(9):
                    nc.tensor.matmul(
                        ps[:],
                        w2s[:, tap * HID:(tap + 1) * HID],
                        conv_rhs(h1, oc * 8, tap, 8),
                        start=(tap == 0),
                        stop=(tap == 8),
                    )
                nc.scalar.activation(
                    ap_of(h2, (oc * 8 + 1) * PW + 1, [[PHW, HID], [PW, 8], [1, W]]),
                    ps[:],
                    RELU,
                    bias=b2s[:],
                )

            with tc.tile_pool(name="gd", bufs=1) as gp:
                guide = [gp.tile([128, PIX], F32R, name=f"gd{m}") for m in range(4)]
                # conv3: HID->C (4 m-chunks), output unpadded [128, 4096]
                for oc in range(8):
                    for mc in range(4):
                        ps = cps.tile([128, 512], F32, tag="cv")
                        for tap in range(9):
                            nc.tensor.matmul(
                                ps[:],
                                w3s[:, tap * C + mc * 128: tap * C + (mc + 1) * 128],
                                conv_rhs(h2, oc * 8, tap, 8),
                                start=(tap == 0),
                                stop=(tap == 8),
                            )
                        nc.scalar.activation(
                            guide[mc][:, oc * 512:(oc + 1) * 512],
                            ps[:],
                            RELU,
                            bias=b3s[:, mc:mc + 1],
                        )

                with tc.tile_pool(name="tfp", bufs=1) as tfp:
                    tf = tfp.tile([RED, PIX], F32R, name="tfeat")
                    for oc in range(8):
                        ps = cps.tile([RED, 512], F32, tag="cv")
                        for cc in range(4):
                            nc.tensor.matmul(
                                ps[:],
                                dw1s[:, cc * RED:(cc + 1) * RED],
                                guide[cc][:, oc * 512:(oc + 1) * 512],
                                start=(cc == 0),
                                stop=(cc == 3),
                            )
                        nc.scalar.activation(
                            tf[:, oc * 512:(oc + 1) * 512], ps[:], RELU,
                            bias=bnshs[:], scale=bnscs[:],
                        )

                    # dck2: RED -> G*TPAD (16 m-chunks of 128 = 2 groups x 64 taps)
                    with tc.tile_pool(name="fsp", bufs=3) as fsp:
                        for mch in range(16):
                            for oc in range(8):
                                ps = cps.tile([128, 512], F32, tag="cv")
                                nc.tensor.matmul(
                                    ps[:],
                                    dw2s[:, mch * 128:(mch + 1) * 128],
                                    tf[:, oc * 512:(oc + 1) * 512],
                                    start=True,
                                    stop=True,
                                )
                                fs = fsp.tile([128, 512], BF16, tag="fs")
                                nc.scalar.activation(fs[:], ps[:], COPY)
                                band, h0 = (oc * 8) // BH, (oc * 8) % BH
                                # fbuf[band, g, t, h, w]: partition (g_loc, t) -> 3-dim dram AP
                                nc.sync.dma_start(
                                    ap_of(
                                        fbuf,
                                        band * (G * TPAD * BH * W)
                                        + 2 * mch * (TPAD * BH * W)
                                        + h0 * W,
                                        [[TPAD * BH * W, 2], [BH * W, TPAD], [1, 512]],
                                    ),
                                    fs[:],
                                )

        # ================= xb build + apply =================
        with tc.tile_pool(name="xbp", bufs=1) as xbp:
            xb_e = xbp.tile([128, GC * CST], BF16)
            xb_o = xbp.tile([128, GC * CST], BF16)
            with tc.tile_pool(name="stg", bufs=1) as stp:
                for cc in range(4):
                    stage = stp.tile([128, 4 * CST], F32, tag="stage")
                    nc.gpsimd.memset(stage[:], 0.0)
                    for band in range(NB):
                        r_lo = max(0, band * BH - 3)
                        r_hi = min(H, band * BH + BH + 3)
                        rows = r_hi - r_lo
                        sro = r_lo - (band * BH - 3)  # stored-row offset
                        for cl in range(4):
                            nc.sync.dma_start(
                                ap_of(
                                    stage,
                                    band * G * (4 * CST) + cl * CST + sro * XW + 3,
                                    [[4 * CST, G], [XW, rows], [1, W]],
                                ),
                                ap_of(
                                    x_in,
                                    (cc * 4 + cl) * PIX + r_lo * W,
                                    [[GC * PIX, G], [W, rows], [1, W]],
                                ),
                            )
                    nc.vector.tensor_copy(
                        xb_e[:, cc * 4 * CST:(cc + 1) * 4 * CST], stage[:]
                    )
                    nc.vector.tensor_copy(
                        ap_of(xb_o, cc * 4 * CST, [[GC * CST, 128], [CST, 4], [XW, XH], [1, XW - 1]]),
                        ap_of(stage, 1, [[4 * CST, 128], [CST, 4], [XW, XH], [1, XW - 1]]),
                    )

            with (
                tc.tile_pool(name="app", bufs=2) as app,
                tc.tile_pool(name="ptp", bufs=3) as ptp,
                tc.tile_pool(name="osb", bufs=2) as op_,
                tc.tile_pool(name="aps", bufs=2, space="PSUM") as aps,
            ):
                for hp in range(8):
                    rt = app.tile([128, 2048], F32, tag="rt")
                    for band in range(NB):
                        nc.sync.dma_start(
                            rt[band * G:(band + 1) * G],
                            ap_of(
                                x_in,
                                band * BH * W + hp * 2 * W,
                                [[GC * PIX, G], [PIX, GC], [W, 2], [1, W]],
                            ),
                        )
                    pso = aps.tile([128, 2048], F32, tag="pso")
                    for tch in range(2):  # tap chunks: 0-31, 32-48
                        t0c, t1c = (0, 32) if tch == 0 else (32, NTAP)
                        ntc = t1c - t0c
                        # ft sbuf layout: (h2, t, w) h-major; fbuf is [band, g, t, h, w]
                        ft = app.tile([128, 2 * 32 * W], BF16, tag="ft")
                        for band in range(NB):
                            for r in range(2):
                                nc.sync.dma_start(
                                    ft[band * G:(band + 1) * G,
                                       r * ntc * W:(r + 1) * ntc * W],
                                    ap_of(
                                        fbuf,
                                        band * (G * TPAD * BH * W)
                                        + t0c * (BH * W)
                                        + (hp * 2 + r) * W,
                                        [[TPAD * BH * W, G], [BH * W, ntc], [1, W]],
                                    ),
                                )
                        for t in range(t0c, t1c):
                            dy, dx = t // K, t % K
                            if dx % 2 == 0:
                                xsrc, bc = xb_e, dx
                            else:
                                xsrc, bc = xb_o, dx - 1
                            in0 = ap_of(
                                xsrc,
                                (hp * 2 + dy) * XW + bc,
                                [[GC * CST, 128], [CST, GC], [XW, 2], [1, W]],
                            )
                            in1 = ap_of(
                                ft,
                                (t - t0c) * W,
                                [[2 * 32 * W, 128], [0, GC], [ntc * W, 2], [1, W]],
                            )
                            pt = ptp.tile([128, 2048], BF16, tag="pt")
                            pout = ap_of(pt, 0, [[2048, 128], [128, GC], [W, 2], [1, W]])
                            nc.vector.tensor_tensor(pout, in0, in1, op=MULT)
                            for j in range(4):
                                nc.tensor.matmul(
                                    pso[:, j * 512:(j + 1) * 512],
                                    idbs[:],
                                    pt[:, j * 512:(j + 1) * 512],
                                    start=(t == 0),
                                    stop=(t == NTAP - 1),
                                )
                    ob = op_.tile([128, 2048], F32, tag="ob")
                    nc.vector.scalar_tensor_tensor(
                        ob[:], pso[:], 1.0, rt[:],
                        op0=MULT, op1=mybir.AluOpType.add,
                    )
                    for band in range(NB):
                        nc.sync.dma_start(
                            ap_of(
                                out,
                                band * BH * W + hp * 2 * W,
                                [[GC * PIX, G], [PIX, GC], [W, 2], [1, W]],
                            ),
                            ob[band * G:(band + 1) * G],
                        )


def prep_weights(inputs):
    """Host-side weight transforms shared by all cores."""
    w1 = np.asarray(inputs["w1"], np.float32)   # [64, 512, 3, 3]
    w2 = np.asarray(inputs["w2"], np.float32)
    w3 = np.asarray(inputs["w3"], np.float32)   # [512, 64, 3, 3]
    dck_w1 = np.asarray(inputs["dck_w1"], np.float32)  # [128, 512, 1, 1]
    dck_w2 = np.asarray(inputs["dck_w2"], np.float32)  # [1568, 128, 1, 1]

    def tapify(w):  # [co, ci, 3, 3] -> [9, ci, co]
        return np.ascontiguousarray(w.transpose(2, 3, 1, 0).reshape(9, w.shape[1], w.shape[0]))

    w1sb = tapify(w1).reshape(9, 4, 128, HID).transpose(2, 0, 1, 3).reshape(128, 9 * 4 * HID)
    w2sb = tapify(w2).transpose(1, 0, 2).reshape(HID, 9 * HID)
    w3sb = tapify(w3).transpose(1, 0, 2).reshape(HID, 9 * C)
    dw1sb = dck_w1.reshape(RED, C).T.reshape(4, 128, RED).transpose(1, 0, 2).reshape(128, 4 * RED)

    bn_g = np.asarray(inputs["bn_gamma"], np.float32)
    bn_b = np.asarray(inputs["bn_beta"], np.float32)
    bn_m = np.asarray(inputs["bn_mean"], np.float32)
    bn_v = np.asarray(inputs["bn_var"], np.float32)
    inv_std = bn_g / np.sqrt(bn_v + 1e-5)
    shift = bn_b - bn_m * inv_std

    dw2 = dck_w2.reshape(G, NTAP, RED)          # [g, t, red]
    dw2p = np.zeros((G, TPAD, RED), np.float32)
    dw2p[:, :NTAP] = dw2
    dw2t = np.ascontiguousarray(dw2p.reshape(G * TPAD, RED).T)  # [red, g*64]

    return {
        "w1t": np.ascontiguousarray(w1sb),
        "b1": np.asarray(inputs["b1"], np.float32).reshape(HID, 1),
        "w2t": np.ascontiguousarray(w2sb),
        "b2": np.asarray(inputs["b2"], np.float32).reshape(HID, 1),
        "w3t": np.ascontiguousarray(w3sb),
        "b3": np.ascontiguousarray(np.asarray(inputs["b3"], np.float32).reshape(4, 128).T),
        "dw1t": np.ascontiguousarray(dw1sb),
        "bnsc": inv_std.reshape(RED, 1),
        "bnsh": shift.reshape(RED, 1),
        "dw2t": dw2t,
        "idb": np.eye(128).astype(ml_dtypes.bfloat16),
    }


_NC_CACHE = {}


def get_nc():
    if "nc" not in _NC_CACHE:
        _NC_CACHE["nc"] = build_nc()
    return _NC_CACHE["nc"]


def kernel(**inputs):
    nc = get_nc()
    wmap = prep_weights(inputs)
    x = np.asarray(inputs["x"], np.float32)
    in_maps = [
        {"x": np.ascontiguousarray(x[i]), "xr": np.ascontiguousarray(x[i]), **wmap}
        for i in range(N_CORES)
    ]
    res = bass_utils.run_bass_kernel_spmd(nc, in_maps, core_ids=list(range(N_CORES)))
    return np.stack([res.results[i]["out"] for i in range(N_CORES)]).astype(np.float32)

